# revision 1
# baseline (speedup 1.0000x reference)
"""DiscreteKDE kernel for 8 Trainium2 NeuronCores.

Full computation:
    Q = 64; H_I = inv(H_bandwidth)
    Z  = (idx[:,None]-idx[None,:]) @ H_I
    KW = (1/sqrt(2pi)) * exp(-0.5 * Z*Z)
    col_sums = concat([X_probs.sum(0), Y_probs.sum(0)])     # (64,)  <- 256MB read
    T  = dot(KW.sum(0), col_sums)
    out = T * jnp.ones((256,256,256))                        # 67MB write

Hard-won structure notes (per-core; stream is the HBM-read roofline,
~300GB/s/core):
  - DGE engine fan-out = largest divisor of an instruction's descriptor
    count that is <= 16. So EVERY DMA instruction here carries a
    multiple-of-16 descriptor count at uniform size: 15 x [128, 4096]
    tiles (16KB descs), tail as [128, 1024] (4KB) + [72, 64] (256B,
    12 engines but only 18KB). A 125-descriptor layout ran on 5 engines
    (125 = 5^3) at 129GB/s; v1's 106-descriptor tail ran on 2.
  - DVE consumes ALL tiles as [128, 512] chunk adds into a [128, 512]
    accumulator (the 8-residual-group space): ~437GB/s, ~82us busy vs a
    ~105us stream. fp32 PE ones-matmuls are 2-pass (~190GB/s) and were
    2.4x too slow to consume the stream alone.
  - ONE PE ones-matmul folds acc -> ps_pe [1, 512]; the dot with
    KW.sum(0) happens in that 512-wide space against an 8x-replicated
    copy (gpsimd mul + XYZWC reduce; gpsimd cannot do X-axis reduces or
    touch PSUM, hence the ACT staging copy).
  - Newton-Schulz inverse of H on PE+ACT (iterating on the negated
    inverse R' = R A R + 2R), alpha chain on gpsimd/ACT (1/x as
    exp(-ln x): DVE is busy streaming, ACT Reciprocal is blocklisted,
    and alpha errors are squared away by the iteration).
  - cross-core sum of the per-core dot d_r: HAND-ROLLED flat all-gather
    via 7 remote_dma_broadcast preps (dest j in slot j so cross-die
    dests ride D2D lanes), one trigger_dma gated on d (add_dep_helper;
    remote preps are "user-synced" so the framework defers NOTHING),
    then a DVE reduce of the [128, 8] gather tile. The recv wait
    (rsem >= 14; each inbound write bumps +2) is spliced into the
    instruction list POST-scheduling: the single-core tile scheduler
    deadlocks on a visible cross-core wait and hoists a depless
    placeholder to the engine's idle front (both tried, both failed).
    Replaces the runtime AllGather whose small-payload latency floor is
    ~20us (trigger delay + mesh walk) with ~2-4us of peer SBUF writes.
  - fill: [128, 2048] tile * T, two 4MB broadcast-AP DMAs on the sync +
    scalar HWDGE rings (~350GB/s write).
"""

import os
import sys

import numpy as np

for _p in ("/opt/trn_rl_repo", "/root/.axon_site/_ro/trn_rl_repo"):
    if os.path.isdir(_p) and _p not in sys.path:
        sys.path.insert(0, _p)

import concourse.bacc as bacc
import concourse.bass as bass
import concourse.mybir as mybir
from concourse.bass_utils import run_bass_kernel_spmd
from concourse.tile import TileContext
from concourse.tile_rust import add_dep_helper

# ---- problem constants (hardcoded per spec) ----
N_TOTAL = 1_000_000
FDIM = 61
HDIM = 3
Q = 64                      # FDIM + HDIM
KGRID = 256
HOUT = 3
NCORES = 8
ROWS_PER_CORE = N_TOTAL // NCORES          # 125000

# ---- tiling: multiple-of-16 descriptor counts everywhere ----
P = 128
G = 64                      # rows/partition/tile -> 16KB descriptors
NT = 15                     # full tiles: 15 * 128 * 64 = 122880 rows
TW = G * Q                  # 4096 f32 = 16KB per partition per tile
CH = 512                    # chunk width = residual-group space
NCHUNK = TW // CH           # 8
MAIN_ROWS = NT * P * G      # 122880
TAILA_G = 16                # [128, 1024]: 2048 rows, 4KB descriptors
TAILA_ROWS = P * TAILA_G    # 2048
TAILB_ROWS = ROWS_PER_CORE - MAIN_ROWS - TAILA_ROWS   # 72
STREAM_BUFS = 10

OUT_TOTAL = KGRID ** HOUT                  # 16_777_216
OUT_PER_CORE = OUT_TOTAL // NCORES         # 2_097_152
FILL_W = 2048
N_FILL = OUT_PER_CORE // (P * FILL_W)      # 8

NEWTON_ITERS = 11
INV_SQRT_2PI = 0.3989422804014327
LN_C = float(np.log(INV_SQRT_2PI))

F32 = mybir.dt.float32
AX = mybir.AxisListType
ALU = mybir.AluOpType
ACT_FN = mybir.ActivationFunctionType


def build_nc(use_remote_exchange=True):
    nc = bacc.Bacc("TRN2", target_bir_lowering=False, debug=False,
                   num_devices=NCORES)

    c_in = nc.dram_tensor("c", [ROWS_PER_CORE, Q], F32, kind="ExternalInput")
    h_in = nc.dram_tensor("h", [Q, Q], F32, kind="ExternalInput")
    out = nc.dram_tensor("o", [OUT_PER_CORE], F32, kind="ExternalOutput")

    idx = np.arange(Q, dtype=np.float64)
    d_const = nc.inline_tensor(
        (idx[:, None] - idx[None, :]).astype(np.float32), "dmat")
    i2_const = nc.inline_tensor(
        (2.0 * np.eye(Q)).astype(np.float32), "i2mat")
    n2_const = nc.inline_tensor(
        (-2.0 * np.eye(Q)).astype(np.float32), "n2mat")

    if not use_remote_exchange:
        cc_in = nc.dram_tensor("cc_in", [1], F32)
        cc_out = nc.dram_tensor("cc_out", [NCORES], F32, addr_space="Shared")

    with TileContext(nc) as tc:
        with (
            tc.tile_pool(name="const", bufs=1) as cpool,
            tc.tile_pool(name="stream", bufs=STREAM_BUFS) as spool,
            tc.tile_pool(name="small", bufs=2) as mpool,
            tc.tile_pool(name="accp", bufs=1, space=bass.MemorySpace.PSUM) as ppool,
            tc.tile_pool(name="psmall", bufs=2, space=bass.MemorySpace.PSUM) as pspool,
        ):
            # ---------- stream DMAs (sync HWDGE ring) ----------
            cv = c_in.ap()[:MAIN_ROWS, :].rearrange(
                "(t p g) q -> t p (g q)", t=NT, p=P, g=G)
            taila_v = c_in.ap()[MAIN_ROWS:MAIN_ROWS + TAILA_ROWS, :].rearrange(
                "(p g) q -> p (g q)", p=P, g=TAILA_G)
            tailb_v = c_in.ap()[MAIN_ROWS + TAILA_ROWS:, :]

            tiles = []
            taila_t = cpool.tile([P, TAILA_G * Q], F32)
            tailb_t = cpool.tile([TAILB_ROWS, Q], F32)
            for t in range(NT):
                st = spool.tile([P, TW], F32, tag="stream")
                if t == NT - 1:
                    # split halves so the final DVE adds chase the DMA
                    nc.sync.dma_start(st[:, :TW // 2], cv[t][:, :TW // 2])
                    nc.sync.dma_start(st[:, TW // 2:], cv[t][:, TW // 2:])
                else:
                    nc.sync.dma_start(st[:], cv[t])
                tiles.append(st)
                if t == 2:
                    # tails early in the ring; dedicated buffers, no
                    # reuse-gating, consumed as soon as they land
                    nc.sync.dma_start(taila_t[:], taila_v)
                    nc.sync.dma_start(tailb_t[:], tailb_v)

            # ---------- constants (gpsimd: DVE streams, ACT does Newton) --
            ones_k = cpool.tile([P, 1], F32)        # lhsT partition-reduce
            nc.gpsimd.memset(ones_k[:], 1.0)
            ones_row = cpool.tile([1, P], F32)      # lhsT bcast scalar->128
            nc.gpsimd.memset(ones_row[:], 1.0)
            ones_q = cpool.tile([Q, 1], F32)        # lhsT 64-part reduce
            nc.gpsimd.memset(ones_q[:], 1.0)
            ones_rq = cpool.tile([1, Q], F32)       # lhsT bcast scalar->64
            nc.gpsimd.memset(ones_rq[:], 1.0)
            lnc = cpool.tile([Q, 1], F32)           # exp bias ln(1/sqrt 2pi)
            nc.gpsimd.memset(lnc[:], LN_C)
            ones_fill = cpool.tile([P, FILL_W], F32)
            nc.gpsimd.memset(ones_fill[:], 1.0)

            # ---------- small inputs (scalar HWDGE ring) ----------
            a_t = cpool.tile([Q, Q], F32)
            nc.scalar.dma_start(a_t[:], h_in.ap())
            d_t = cpool.tile([Q, Q], F32)
            nc.scalar.dma_start(d_t[:], d_const.ap())
            i2_t = cpool.tile([Q, Q], F32)
            nc.scalar.dma_start(i2_t[:], i2_const.ap())
            n2_t = cpool.tile([Q, Q], F32)
            nc.scalar.dma_start(n2_t[:], n2_const.ap())

            # ---------- exchange state ----------
            if use_remote_exchange:
                rsem = nc.alloc_semaphore("xg_rsem")
                lsem = nc.alloc_semaphore("xg_lsem")
                ag = cpool.tile([P, NCORES], F32)    # gathered d_r
                db = cpool.tile([P, 1], F32)         # my d on 128 parts

            # ---------- alpha chain (gpsimd + ACT + PE) ----------
            # tmp_qq = A * 2I is diagonal => 2*trace = full-tensor sum,
            # exactly gpsimd's XYZWC (partition-inclusive) reduce.
            tmp_qq = mpool.tile([Q, Q], F32, tag="qq")
            nc.gpsimd.tensor_mul(tmp_qq[:], a_t[:], i2_t[:])
            tr_s = mpool.tile([1, 1], F32, tag="q1")
            nc.gpsimd.tensor_reduce(tr_s[:], tmp_qq[:], axis=AX.XYZWC,
                                    op=ALU.add)
            ln_t = mpool.tile([1, 1], F32, tag="s11ln")
            nc.scalar.activation(ln_t[:], tr_s[:], ACT_FN.Ln)
            tr2 = mpool.tile([1, 1], F32, tag="s11")
            nc.scalar.activation(tr2[:], ln_t[:], ACT_FN.Exp, scale=-1.0)
            ps_a = pspool.tile([Q, 1], F32, tag="ps_small")
            nc.tensor.matmul(ps_a[:], ones_rq[:], tr2[:])     # bcast->(64,1)
            al64 = mpool.tile([Q, 1], F32, tag="q1b")
            nc.scalar.activation(al64[:], ps_a[:], ACT_FN.Copy)

            # ---------- DVE stream: chunk adds into acc [128, 512] --------
            acc = cpool.tile([P, CH], F32)
            last_dve = [None]
            dve_n = [0]

            def consume_chunks(tile, ncols):
                for b in range(ncols // CH):
                    sl = tile[:, b * CH:(b + 1) * CH]
                    if dve_n[0] == 0:
                        last_dve[0] = nc.vector.tensor_copy(acc[:], sl)
                    else:
                        last_dve[0] = nc.vector.tensor_add(acc[:], acc[:], sl)
                    dve_n[0] += 1

            consume_chunks(tiles[0], TW)
            consume_chunks(tiles[1], TW)
            consume_chunks(tiles[2], TW)
            consume_chunks(taila_t, TAILA_G * Q)
            # tailB: 72 rows of 64 cols land on acc group 0 (q-aligned)
            last_dve[0] = nc.vector.tensor_add(
                acc[:TAILB_ROWS, :Q], acc[:TAILB_ROWS, :Q], tailb_t[:])
            dve_n[0] += 1
            for t in range(3, NT):
                consume_chunks(tiles[t], TW)

            # ---------- Newton-Schulz on PE + ACT (PE is free now) --------
            s_cur = mpool.tile([Q, Q], F32, tag="newton")
            nc.gpsimd.tensor_scalar_mul(s_cur[:], n2_t[:], al64[:])
            for it in range(NEWTON_ITERS):
                ps_y = pspool.tile([Q, Q], F32, tag="ps_qq")
                nc.tensor.matmul(ps_y[:], a_t[:], s_cur[:])       # A @ R
                y_sb = mpool.tile([Q, Q], F32, tag="newton_y")
                nc.scalar.activation(y_sb[:], ps_y[:], ACT_FN.Copy)
                ps_x = pspool.tile([Q, Q], F32, tag="ps_qq")
                nc.tensor.matmul(ps_x[:], s_cur[:], y_sb[:],
                                 start=True, stop=False)          # R A R
                nc.tensor.matmul(ps_x[:], s_cur[:], i2_t[:],
                                 start=False, stop=True)          # + 2 R
                s_nxt = mpool.tile([Q, Q], F32, tag="newton")
                nc.scalar.activation(s_nxt[:], ps_x[:], ACT_FN.Copy)
                s_cur = s_nxt
            # Z = D.T @ (-H^-1) up to sign; KW = exp(-Z^2/2 + ln c)
            ps_z = pspool.tile([Q, Q], F32, tag="ps_qq")
            nc.tensor.matmul(ps_z[:], d_t[:], s_cur[:])
            z2 = mpool.tile([Q, Q], F32, tag="qq2")
            nc.scalar.square(z2[:], ps_z[:])
            kw = mpool.tile([Q, Q], F32, tag="qq3")
            nc.scalar.activation(kw[:], z2[:], ACT_FN.Exp,
                                 bias=lnc[:], scale=-0.5)
            ps_s = pspool.tile([1, Q], F32, tag="ps_small")
            nc.tensor.matmul(ps_s[:], ones_q[:], kw[:])           # KW.sum(0)
            # replicate KW.sum(0) 8x into the 512-wide residual space
            s_rep = mpool.tile([1, NCHUNK * Q], F32, tag="vec2")
            s_rep_v = s_rep[:].rearrange("p (g q) -> p g q", g=NCHUNK, q=Q)
            ps_s_b = ps_s[:].unsqueeze(1).broadcast_to([1, NCHUNK, Q])
            nc.scalar.activation(s_rep_v, ps_s_b, ACT_FN.Copy)

            # ---------- close the stream: acc -> ps_pe [1, 512] ----------
            ps_pe = ppool.tile([1, CH], F32)
            nc.tensor.matmul(ps_pe[:], ones_k[:], acc[:])

            # ---------- local dot d = <KW.sum(0), col sums> ----------
            pe_sb = mpool.tile([1, NCHUNK * Q], F32, tag="vec0")
            nc.scalar.activation(pe_sb[:], ps_pe[:], ACT_FN.Copy)
            dprod = mpool.tile([1, NCHUNK * Q], F32, tag="vec2b")
            nc.gpsimd.tensor_mul(dprod[:], s_rep[:], pe_sb[:])
            d_loc = mpool.tile([1, 1], F32, tag="s11d")
            nc.gpsimd.tensor_reduce(d_loc[:], dprod[:], axis=AX.XYZWC,
                                    op=ALU.add)

            if use_remote_exchange:
                # broadcast d to 128 partitions, then fire the peer writes
                ps_b = pspool.tile([P, 1], F32, tag="ps_small")
                nc.tensor.matmul(ps_b[:], ones_row[:], d_loc[:])
                db_cp = nc.scalar.activation(db[:], ps_b[:], ACT_FN.Copy)
                nc.gpsimd.tensor_copy(ag[:, 0:1], db[:])          # self slot
                # preps are descriptor-gen only; the SDMA reads db when the
                # trigger fires, so only the trigger needs the data dep.
                for j in range(1, NCORES):
                    rd = [None] * NCORES
                    rd[j] = (0, j)
                    nc.gpsimd.remote_dma_broadcast(
                        ag[:, j:j + 1], db[:],
                        remote_sem=rsem, local_sem=lsem, rdests=rd)
                trig = nc.gpsimd.trigger_dma(count=None)
                add_dep_helper(trig.ins, db_cp.ins, sync=True,
                               reason="fire peer writes only once d final")
                tb = mpool.tile([P, 1], F32, tag="tb")
                red = nc.vector.tensor_reduce(tb[:], ag[:], axis=AX.X,
                                              op=ALU.add)
                # pin the reduce after DVE's stream adds so the post-
                # scheduling recv wait spliced before it cannot park DVE
                # mid-stream (that would deadlock the exchange globally)
                add_dep_helper(red.ins, last_dve[0].ins, sync=True,
                               reason="reduce after local stream adds")
                # registered directly so compile() still emits the prelude
                # AllGather that synchronizes the 8 kernel launches
                assert nc._bir_kernel_barrier_sem is not None
                nc._bir_kernel_barrier_sem_replica_groups.append(
                    set(range(NCORES)))
            else:
                nc.sync.dma_start(cc_in.ap(), d_loc[:])
                nc.gpsimd.collective_compute(
                    "AllGather", ALU.bypass,
                    replica_groups=[list(range(NCORES))],
                    ins=[cc_in.ap()], outs=[cc_out.ap()],
                )
                gath = mpool.tile([1, NCORES], F32, tag="gath")
                nc.sync.dma_start(gath[:], cc_out.ap())
                t_sc = mpool.tile([1, 1], F32, tag="s11c")
                nc.vector.tensor_reduce(t_sc[:], gath[:], axis=AX.X,
                                        op=ALU.add)
                ps_b = pspool.tile([P, 1], F32, tag="ps_small")
                nc.tensor.matmul(ps_b[:], ones_row[:], t_sc[:])
                tb = mpool.tile([P, 1], F32, tag="tb")
                nc.scalar.activation(tb[:], ps_b[:], ACT_FN.Copy)

            # ---------- fill ----------
            fill = cpool.tile([P, FILL_W], F32)
            nc.vector.tensor_scalar_mul(fill[:], ones_fill[:], tb[:])
            half = N_FILL // 2
            ovh = out.ap().rearrange("(h j p f) -> h p j f",
                                     h=2, p=P, f=FILL_W)
            fill_b = fill[:].unsqueeze(1).broadcast_to([P, half, FILL_W])
            nc.sync.dma_start(ovh[0], fill_b)
            nc.scalar.dma_start(ovh[1], fill_b)

    if use_remote_exchange:
        # Recv wait, spliced in POST-scheduling directly before the reduce
        # (the scheduler deadlocks on a visible cross-core wait and hoists
        # a depless placeholder to the engine's idle front). Each inbound
        # peer write bumps rsem by 16 // n_dests = 2, so 7 peers => 14.
        w = nc.vector.wait_ge(rsem, 2 * (NCORES - 1))
        wins = w.ins
        fn = nc.m.functions[0]
        for blk in fn.blocks:
            il = blk.instructions
            for i in range(len(il)):
                if il[i] is wins:
                    del il[i]
                    break
        placed = False
        for blk in fn.blocks:
            il = blk.instructions
            for i in range(len(il)):
                if il[i] is red.ins:
                    il.insert(i, wins)
                    placed = True
                    break
            if placed:
                break
        assert placed, "could not splice recv wait before reduce"

    nc.compile()
    return nc


_NC_CACHE = None


def _get_nc():
    global _NC_CACHE
    if _NC_CACHE is None:
        _NC_CACHE = build_nc()
    return _NC_CACHE


def run(X_probs, Y_probs, H_bandwidth, trace=False, trace_kwargs=None):
    X = np.asarray(X_probs, dtype=np.float32).reshape(NCORES, ROWS_PER_CORE, FDIM)
    Y = np.asarray(Y_probs, dtype=np.float32).reshape(NCORES, ROWS_PER_CORE, HDIM)
    H = np.ascontiguousarray(np.asarray(H_bandwidth, dtype=np.float32))

    C = np.empty((NCORES, ROWS_PER_CORE, Q), dtype=np.float32)
    C[:, :, :FDIM] = X
    C[:, :, FDIM:] = Y

    nc = _get_nc()
    in_maps = [{"c": C[i], "h": H} for i in range(NCORES)]
    res = run_bass_kernel_spmd(nc, in_maps, list(range(NCORES)),
                               trace=trace, **(trace_kwargs or {}))
    full = np.concatenate([res.results[i]["o"] for i in range(NCORES)])
    return full.reshape((KGRID,) * HOUT), res


def kernel(X_probs, Y_probs, H_bandwidth, K, H_out):
    assert int(K) == KGRID and int(H_out) == HOUT
    out, _ = run(X_probs, Y_probs, H_bandwidth, trace=False)
    return out

